# revision 13
# baseline (speedup 1.0000x reference)
"""Trainium2 Bass kernel for nn_AdvancedPassPredictionModel.

Sharding: data-parallel over batch — core c computes batch element c
(B=8, 8 cores). No collectives. Each core runs the full transformer
block for its [S=1024, D=1024] slice.

Per-core dataflow (all matmul operands bf16, fp32 PSUM accumulation,
fp32 residual stream):

  x --LN1--> xn (bf16 row) --xbar T--> xnT (T4 layout)
  QT = wq'^T @ xnT (column), KT likewise, V = xnT^T @ wv' (row, with an
  appended ones-column per head so attn@V also yields the softmax
  denominator in row 64).
  scoresT[k,q] per head via lhsT=KT_h, rhs=QT_h (K=dk contraction);
  softmax without max-subtraction (inputs are LN'd, scores are small):
  exp on ScalarE (scale=1/sqrt(dk) fused) -> expT bf16; attn@V via
  lhsT=V'_h -> psum rows [0:dk]=ctxT, row dk=sumexp; normalize+importance
  via DVE mult with a gpsimd partition-broadcast row.
  Importance MLP from pooled mean (reduced from xnT) -> per-head scalars.
  out-proj row-layout + residual add into x (f32, in place).
  Tactical branch identical in structure (dk=32, ones at col 32).
  gate = sigmoid([x2T|tfT]^T @ tg_w), fused = x2 + gate*tproj, LN2 -> x3,
  LN3 -> xn3 --T--> xn3T, FFN (h1T column with fused relu+bias,
  h2 row accumulated over 32 k-tiles) + residual -> y.

Weights are host-side prepacked (bf16 cast, LN-gamma folding, SBUF tile
order) so the device program contains no weight-format work.
"""

import math
from contextlib import ExitStack

import numpy as np
import ml_dtypes

import concourse.bass as bass
import concourse.mybir as mybir
import concourse.tile as tile
from concourse import bacc
from concourse.masks import make_identity

F32 = mybir.dt.float32
BF16 = mybir.dt.bfloat16
AF = mybir.ActivationFunctionType
OP = mybir.AluOpType
AX = mybir.AxisListType

BF = ml_dtypes.bfloat16

P = 128
B, S, D, H, DK = 8, 1024, 1024, 16, 64
TD, TH, TDK = 256, 8, 32
FF = 4096
EPS = 1e-5
ST = S // P          # 8 s-tiles
KD = D // P          # 8 d k-tiles
KTD = TD // P        # 2
MF = FF // P         # 32 ff m-tiles
NCH = 512            # matmul free-dim chunk
SCALE_SA = 1.0 / math.sqrt(DK)
SCALE_TA = 1.0 / math.sqrt(TDK)

N_CORES = 8


def ts(i, n):
    return slice(i * n, (i + 1) * n)


def _pack_weights(params):
    """Host-side prepack: bf16 casts, LN-gamma folds, SBUF tile order.

    Returns (dram_inputs: dict name->np.ndarray, flags: dict).
    """
    p = {k: np.asarray(v, np.float32) for k, v in params.items()
         if not isinstance(v, dict)}
    sa = {k: np.asarray(v, np.float32) for k, v in params['sa'].items()}
    ta = {k: np.asarray(v, np.float32) for k, v in params['ta'].items()}

    def lhsT_pack(w, mt):
        # w [K, M] -> [mt, 128(p=K inner), K//128(k), 128(j=M inner)]
        K, M = w.shape
        kt = K // P
        assert mt * P == M
        return np.ascontiguousarray(
            w.reshape(kt, P, mt, P).transpose(2, 1, 0, 3)).astype(BF)

    def rhs_pack(w, nch=NCH):
        # w [K, N] -> [N//nch, 128(p), K//128(k), nch]  (partition-major,
        # matching the [128, kt, nch] SBUF tile element order)
        K, N = w.shape
        kt, nt = K // P, N // nch
        return np.ascontiguousarray(
            w.reshape(kt, P, nt, nch).transpose(2, 1, 0, 3)).astype(BF)

    def col(v):
        # [D'] -> [128, D'//128] column layout (d = k*128 + p)
        n = v.shape[0] // P
        return np.ascontiguousarray(v.reshape(n, P).T).astype(np.float32)

    g1, b1 = p['n1_g'], p['n1_b']
    g2, b2 = p['n2_g'], p['n2_b']
    g3, b3 = p['n3_g'], p['n3_b']

    d = {}
    flags = {}

    # self-attention
    d['wq_l'] = lhsT_pack(g1[:, None] * sa['wq'], KD)
    d['wk_l'] = lhsT_pack(g1[:, None] * sa['wk'], KD)
    d['wv_r'] = rhs_pack(g1[:, None] * sa['wv'])
    d['ow_r'] = rhs_pack(sa['ow'])
    d['iw1_l'] = lhsT_pack(sa['iw1'], (D // 2) // P)
    d['iw2_l'] = np.ascontiguousarray(
        sa['iw2'].reshape(4, P, H)).astype(BF)            # [4k, 128, 16]
    qb = b1 @ sa['wq']
    kb = b1 @ sa['wk']
    vb = b1 @ sa['wv']
    flags['qb'] = bool(np.any(qb)); d['qb_c'] = col(qb)
    flags['kb'] = bool(np.any(kb)); d['kb_c'] = col(kb)
    flags['vb'] = bool(np.any(vb)); d['vb_row'] = vb[None, :].astype(BF)
    d['ib1_c'] = col(sa['ib1'])
    d['ib2_c'] = sa['ib2'][:, None].astype(np.float32)     # [16, 1]
    flags['ob'] = bool(np.any(sa['ob'])); d['ob_row'] = sa['ob'][None, :].astype(BF)
    flags['g1pool'] = not np.allclose(g1, 1.0)
    flags['b1pool'] = bool(np.any(b1))
    d['g1_c'] = col(g1)
    d['b1_c'] = col(b1)

    # tactical attention
    d['tq_l'] = lhsT_pack(ta['wq'], KTD)
    d['tk_l'] = lhsT_pack(ta['wk'], KTD)
    d['tv_r'] = rhs_pack(ta['wv'], nch=TD)                 # [1, 2, 128, 256]
    d['tow_r'] = rhs_pack(ta['ow'], nch=TD)
    d['ti1_l'] = lhsT_pack(ta['iw1'], 1)                   # [1, 128, 2, 128]
    d['ti2_l'] = np.ascontiguousarray(ta['iw2']).astype(BF)  # [128, 8]
    d['ti1_c'] = col(ta['ib1'])                            # [128, 1]
    d['ti2_c'] = ta['ib2'][:, None].astype(np.float32)     # [8, 1]
    flags['tob'] = bool(np.any(ta['ob'])); d['tob_row'] = ta['ob'][None, :].astype(BF)

    # fusion
    d['tp_r'] = rhs_pack(p['tp_w'])
    flags['tpb'] = bool(np.any(p['tp_b'])); d['tpb_row'] = p['tp_b'][None, :].astype(BF)
    d['tg_r'] = rhs_pack(p['tg_w'])
    flags['tgb'] = bool(np.any(p['tg_b'])); d['tgb_row'] = p['tg_b'][None, :].astype(BF)
    flags['g2'] = not np.allclose(g2, 1.0)
    flags['b2'] = bool(np.any(b2))
    d['g2_r'] = g2[None, :].astype(np.float32)
    d['b2_r'] = b2[None, :].astype(np.float32)

    # FFN
    d['w1_l'] = lhsT_pack(g3[:, None] * p['ff_w1'], MF)
    d['w2_r'] = rhs_pack(p['ff_w2'])
    d['h1b_c'] = np.ascontiguousarray(
        (b3 @ p['ff_w1'] + p['ff_b1']).reshape(MF, P).T).astype(np.float32)
    flags['fb2'] = bool(np.any(p['ff_b2'])); d['fb2_row'] = p['ff_b2'][None, :].astype(BF)

    # drop unused conditional tensors so the NEFF input list stays exact
    if not flags['qb']: d.pop('qb_c')
    if not flags['kb']: d.pop('kb_c')
    if not flags['vb']: d.pop('vb_row')
    if not flags['ob']: d.pop('ob_row')
    if not (flags['g1pool'] or flags['b1pool']):
        d.pop('g1_c'); d.pop('b1_c')
    if not flags['tob']: d.pop('tob_row')
    if not flags['tpb']: d.pop('tpb_row')
    if not flags['tgb']: d.pop('tgb_row')
    if not flags['g2']: d.pop('g2_r')
    if not flags['b2']: d.pop('b2_r')
    if not flags['fb2']: d.pop('fb2_row')
    return d, flags


def _layernorm_rowtiles(nc, pool, x_ap, out_ap, eps_ap=None):
    """LN over the free dim for each [128, D] row tile of x_ap [128, T, D].

    Writes normalized (x-m)*rstd into out_ap (same [128, T, D] shape).
    gamma/beta are NOT applied here (folded into consumers by callers).
    """
    T = x_ap.shape[1]
    Dd = x_ap.shape[2]
    nsub = Dd // 512
    for t in range(T):
        stats = pool.tile([P, nsub, 6], F32, tag="ln_stats")
        xin = x_ap[:, t, :].rearrange("p (a b) -> p a b", a=nsub)
        for ssub in range(nsub):
            nc.vector.bn_stats(stats[:, ssub, :], xin[:, ssub, :])
        mv = pool.tile([P, 2], F32, tag="ln_mv")
        nc.vector.bn_aggr(mv[:], stats[:])
        sd = pool.tile([P, 1], F32, tag="ln_sd")
        nc.scalar.activation(sd[:], mv[:, 1:2], AF.Sqrt, bias=eps_ap[:])
        rstd = pool.tile([P, 1], F32, tag="ln_rstd")
        nc.vector.reciprocal(rstd[:], sd[:])
        nmr = pool.tile([P, 1], F32, tag="ln_nmr")
        nc.vector.tensor_tensor(nmr[:], mv[:, 0:1], rstd[:], OP.mult)
        nc.vector.tensor_scalar_mul(nmr[:], nmr[:], -1.0)
        # out = x*rstd + (-m*rstd)
        nc.vector.tensor_scalar(out_ap[:, t, :], x_ap[:, t, :],
                                rstd[:], nmr[:], OP.mult, OP.add)


def build(flags, dbg=False):
    """Build the per-core Bass program. Returns (nc, input_names)."""
    nc = bacc.Bacc("TRN2", target_bir_lowering=False, debug=False)
    dbg_out = {}
    def dbg_dump(nc_, name, ap):
        if not dbg:
            return
        t = nc_.dram_tensor("dbg_" + name, list(ap.shape),
                            ap.dtype, kind="ExternalOutput").ap()
        nc_.sync.dma_start(t, ap)
        dbg_out[name] = t

    x_d = nc.dram_tensor("x", [S, D], F32, kind="ExternalInput").ap()
    tc_d = nc.dram_tensor("tc", [S, TD], F32, kind="ExternalInput").ap()
    y_d = nc.dram_tensor("y", [S, D], F32, kind="ExternalOutput").ap()

    w = {}
    def dram(name, shape, dt=BF16):
        w[name] = nc.dram_tensor(name, list(shape), dt, kind="ExternalInput").ap()

    dram('wq_l', [KD, P, KD, P]); dram('wk_l', [KD, P, KD, P])
    dram('wv_r', [2, P, KD, NCH]); dram('ow_r', [2, P, KD, NCH])
    dram('iw1_l', [4, P, KD, P]); dram('iw2_l', [4, P, H])
    dram('ib1_c', [P, 4], F32); dram('ib2_c', [H, 1], F32)
    dram('tq_l', [KTD, P, KTD, P]); dram('tk_l', [KTD, P, KTD, P])
    dram('tv_r', [1, P, KTD, TD]); dram('tow_r', [1, P, KTD, TD])
    dram('ti1_l', [1, P, KTD, P]); dram('ti2_l', [P, TH])
    dram('ti1_c', [P, 1], F32); dram('ti2_c', [TH, 1], F32)
    dram('tp_r', [2, P, KTD, NCH]); dram('tg_r', [2, P, KD + KTD, NCH])
    dram('w1_l', [MF, P, KD, P]); dram('w2_r', [2, P, MF, NCH])
    dram('h1b_c', [P, MF], F32)
    if flags['qb']: dram('qb_c', [P, KD], F32)
    if flags['kb']: dram('kb_c', [P, KD], F32)
    if flags['vb']: dram('vb_row', [1, D])
    if flags['ob']: dram('ob_row', [1, D])
    if flags['g1pool'] or flags['b1pool']:
        dram('g1_c', [P, KD], F32); dram('b1_c', [P, KD], F32)
    if flags['tob']: dram('tob_row', [1, TD])
    if flags['tpb']: dram('tpb_row', [1, D])
    if flags['tgb']: dram('tgb_row', [1, D])
    if flags['g2']: dram('g2_r', [1, D], F32)
    if flags['b2']: dram('b2_r', [1, D], F32)
    if flags['fb2']: dram('fb2_row', [1, D])

    with tile.TileContext(nc) as tc, ExitStack() as LL:
        # ---- long-lived pools ----
        const = LL.enter_context(tc.tile_pool(name="const", bufs=1))
        big = LL.enter_context(tc.tile_pool(name="big", bufs=1))
        lnp = LL.enter_context(tc.tile_pool(name="lnp", bufs=4))
        wl = LL.enter_context(tc.tile_pool(name="wl", bufs=3))
        ypool = LL.enter_context(tc.tile_pool(name="yp", bufs=2))
        ps = LL.enter_context(tc.tile_pool(name="ps", bufs=8, space="PSUM"))

        def psum(part=P, n=NCH):
            t_ = ps.tile([P, NCH], F32, tag="ps", name="ps")
            return t_[:part, :n]

        x_sb = big.tile([P, ST, D], F32, tag="x")          # residual stream
        tfT = big.tile([P, ST, KTD, P], BF16, tag="tfT")

        ident = const.tile([16, 16], F32)
        make_identity(nc, ident[:])
        ones_row = const.tile([1, P], BF16)
        nc.vector.memset(ones_row[:], 1.0)
        eps_ap = const.tile([P, 1], F32, tag="eps")
        nc.vector.memset(eps_ap[:], EPS)

        cbias = {}
        for nm in ('ib1_c', 'ib2_c', 'ti1_c', 'ti2_c', 'h1b_c',
                   'qb_c', 'kb_c', 'g1_c', 'b1_c'):
            if nm in w:
                t_ = const.tile(list(w[nm].shape), F32, tag=nm)
                nc.sync.dma_start(t_[:], w[nm][:])
                cbias[nm] = t_
        rows = {}
        for nm in ('vb_row', 'ob_row', 'tob_row', 'tpb_row', 'tgb_row',
                   'fb2_row'):
            if nm in w:
                t_ = const.tile(list(w[nm].shape), BF16, tag=nm)
                nc.sync.dma_start(t_[:], w[nm][:])
                rows[nm] = t_
        for nm in ('g2_r', 'b2_r'):
            if nm in w:
                t_ = const.tile([1, D], F32, tag=nm)
                nc.sync.dma_start(t_[:], w[nm][:])
                rows[nm] = t_

        def bias_mm(psum_ap, row_ap, nslice):
            # accumulate ones^T @ bias_row chunk into an open psum group
            nc.tensor.matmul(psum_ap, ones_row[:, 0:psum_ap.shape[0]],
                             row_ap[:, nslice], start=False, stop=True,
                             skip_group_check=True)

        # ---- input load + LN1 + transposes ----
        nc.sync.dma_start(x_sb[:], x_d.rearrange("(t p) d -> p t d", p=P))

        rowtmp = big.tile([P, ST, D], BF16, tag="rowtmp")  # xn1
        _layernorm_rowtiles(nc, lnp, x_sb[:], rowtmp[:], eps_ap)
        T4 = big.tile([P, ST, KD, P], BF16, tag="T4")      # xnT
        for t in range(ST):
            nc.sync.dma_start_transpose(T4[:, t, :, :], rowtmp[:, t, :])
        dbg_dump(nc, "xn1", rowtmp[:])
        dbg_dump(nc, "T4", T4[:])

        tcT = big.tile([P, ST, KTD, P], BF16, tag="tcT")
        with tc.tile_pool(name="tcp", bufs=1) as tcp:
            tc_f = tcp.tile([P, ST, TD], F32, tag="tc_f")
            nc.sync.dma_start(tc_f[:], tc_d.rearrange("(t p) d -> p t d", p=P))
            tc16 = tcp.tile([P, ST, TD], BF16, tag="tc16")
            nc.vector.tensor_copy(tc16[:], tc_f[:])
            for t in range(ST):
                nc.sync.dma_start_transpose(tcT[:, t, :, :], tc16[:, t, :])

        def rhs_chunk(T4_ap, nn, k):
            # [128, 4, 128] strided N=512 moving operand: s-chunk nn, k-tile k
            return T4_ap[:, 4 * nn:4 * nn + 4, k, :]

        # ---- importance MLPs (self + tactical) ----
        def importance(T4_ap, kd, iw1_t, iw2_t, ib1_t, ib2_t, nheads,
                       gcol, bcol, tag):
            # pooled mean from transposed activations
            r1 = lnp.tile([P, ST, kd], F32, tag=f"r1{tag}")
            nc.vector.reduce_sum(r1[:], T4_ap[:], axis=AX.X)
            pooled = lnp.tile([P, kd], F32, tag=f"pool{tag}")
            nc.vector.reduce_sum(pooled[:], r1[:].transpose([0, 2, 1]), axis=AX.X)
            nc.vector.tensor_scalar_mul(pooled[:], pooled[:], 1.0 / S)
            if gcol is not None:
                nc.vector.tensor_tensor(pooled[:], pooled[:], gcol[:], OP.mult)
            if bcol is not None:
                nc.vector.tensor_tensor(pooled[:], pooled[:], bcol[:], OP.add)
            pooled16 = lnp.tile([P, kd], BF16, tag=f"pool16{tag}")
            nc.vector.tensor_copy(pooled16[:], pooled[:])
            # hidT [hdim] column
            hm = iw1_t.shape[0]
            hidT = lnp.tile([P, hm, 1], BF16, tag=f"hid{tag}")
            wtag = "wl" if kd == KD else "wlt"
            for m in range(hm):
                w1t = wl.tile([P, kd, P], BF16, tag=wtag)
                nc.sync.dma_start(w1t[:], iw1_t[m])
                hp = psum(P, 1)
                for k in range(kd):
                    nc.tensor.matmul(hp, w1t[:, k, :], pooled16[:, k:k + 1],
                                     start=(k == 0), stop=(k == kd - 1))
                nc.scalar.activation(hidT[:, m, :], hp, AF.Relu,
                                     bias=ib1_t[:, m:m + 1])
            # logitsT [nheads, 1]; iw2_t dram: [kt, 128, nheads] or [128, nheads]
            lp = psum(nheads, 1)
            if len(iw2_t.shape) == 3:
                kt = iw2_t.shape[0]
                for k in range(kt):
                    i2k = wl.tile([P, nheads], BF16, tag=f"iw2{tag}")
                    nc.sync.dma_start(i2k[:], iw2_t[k])
                    nc.tensor.matmul(lp, i2k[:], hidT[:, k, :],
                                     start=(k == 0), stop=(k == kt - 1))
            else:
                i2 = wl.tile([P, iw2_t.shape[-1]], BF16, tag=f"iw2{tag}")
                nc.sync.dma_start(i2[:], iw2_t[:])
                nc.tensor.matmul(lp, i2[:], hidT[:, 0, :], start=True, stop=True)
            logits = lnp.tile([nheads, 1], F32, tag=f"lg{tag}")
            nc.scalar.activation(logits[:], lp, AF.Identity, bias=ib2_t[:])
            # transpose -> row, softmax
            lrow_ps = psum(1, nheads)
            nc.tensor.transpose(lrow_ps, logits[:], ident[0:nheads, 0:nheads])
            erow = lnp.tile([1, nheads], F32, tag=f"erow{tag}")
            nc.scalar.activation(erow[:], lrow_ps, AF.Exp)
            ssum = lnp.tile([1, 1], F32, tag=f"ssum{tag}")
            nc.vector.reduce_sum(ssum[:], erow[:], axis=AX.X)
            rcp = lnp.tile([1, 1], F32, tag=f"rcp{tag}")
            nc.vector.reciprocal(rcp[:], ssum[:])
            imp = const.tile([1, nheads], F32, tag=f"imp{tag}")
            nc.vector.tensor_scalar_mul(imp[:], erow[:], rcp[0:1, 0:1])
            return imp

        imp_sa = importance(T4[:], KD, w['iw1_l'], w['iw2_l'],
                            cbias['ib1_c'], cbias['ib2_c'], H,
                            cbias.get('g1_c'), cbias.get('b1_c'), "sa")
        imp_ta = importance(tcT[:], KTD, w['ti1_l'], w['ti2_l'],
                            cbias['ti1_c'], cbias['ti2_c'], TH,
                            None, None, "ta")
        dbg_dump(nc, "tcT", tcT[:])
        dbg_dump(nc, "imp_sa", imp_sa[:])
        dbg_dump(nc, "imp_ta", imp_ta[:])

        # ---- phase 1: attention ----
        P1 = ExitStack()
        p1 = P1.enter_context(tc.tile_pool(name="p1", bufs=1))
        p1w = P1.enter_context(tc.tile_pool(name="p1w", bufs=2))
        expp = P1.enter_context(tc.tile_pool(name="expp", bufs=2))
        rbp = P1.enter_context(tc.tile_pool(name="rbp", bufs=2))

        VP = p1.tile([P, ST, H * (DK + 1)], BF16, tag="VP")
        nc.vector.memset(VP[:, :, DK::DK + 1], 1.0)
        VPt = p1.tile([P, ST, TH * (TDK + 1)], BF16, tag="VPt")
        nc.vector.memset(VPt[:, :, TDK::TDK + 1], 1.0)
        ctxT = p1.tile([P, KD, S], BF16, tag="ctxT")
        ctxTt = p1.tile([P, KTD, S], BF16, tag="ctxTt")
        tfrow = p1.tile([P, ST, TD], BF16, tag="tfrow")

        # tactical QKV (small, emitted first so PE has early work)
        QTt = p1.tile([P, KTD, S], BF16, tag="QTt")
        KTt = p1.tile([P, KTD, S], BF16, tag="KTt")
        for m in range(KTD):
            wqt = wl.tile([P, KTD, P], BF16, tag="wlt")
            nc.sync.dma_start(wqt[:], w['tq_l'][m])
            wkt = wl.tile([P, KTD, P], BF16, tag="wlt")
            nc.sync.dma_start(wkt[:], w['tk_l'][m])
            for nn in range(2):
                qp = psum()
                for k in range(KTD):
                    nc.tensor.matmul(qp, wqt[:, k, :], rhs_chunk(tcT, nn, k),
                                     start=(k == 0), stop=(k == KTD - 1))
                nc.scalar.copy(QTt[:, m, ts(nn, NCH)], qp)
                kp = psum()
                for k in range(KTD):
                    nc.tensor.matmul(kp, wkt[:, k, :], rhs_chunk(tcT, nn, k),
                                     start=(k == 0), stop=(k == KTD - 1))
                nc.vector.tensor_copy(KTt[:, m, ts(nn, NCH)], kp)
        tvw = p1w.tile([P, KTD, TD], BF16, tag="tvw")
        nc.sync.dma_start(tvw[:], w['tv_r'][0])
        for t in range(ST):
            vp_ = psum(P, TD)
            for k in range(KTD):
                nc.tensor.matmul(vp_, tcT[:, t, k, :], tvw[:, k, :],
                                 start=(k == 0), stop=(k == KTD - 1))
            dst = VPt[:, t, :].rearrange("p (h e) -> p h e", e=TDK + 1)[:, :, 0:TDK]
            nc.vector.tensor_copy(
                dst, vp_.rearrange("p (h e) -> p h e", e=TDK))

        # self QKV
        QKp = ExitStack()
        qk = QKp.enter_context(tc.tile_pool(name="qk", bufs=1))
        QT = qk.tile([P, KD, S], BF16, tag="QT")
        KT = qk.tile([P, KD, S], BF16, tag="KT")
        for m in range(KD):
            wq = wl.tile([P, KD, P], BF16, tag="wl")
            nc.sync.dma_start(wq[:], w['wq_l'][m])
            wk = wl.tile([P, KD, P], BF16, tag="wl")
            nc.sync.dma_start(wk[:], w['wk_l'][m])
            for nn in range(2):
                qp = psum()
                for k in range(KD):
                    nc.tensor.matmul(qp, wq[:, k, :], rhs_chunk(T4, nn, k),
                                     start=(k == 0), stop=(k == KD - 1))
                if flags['qb']:
                    nc.scalar.activation(QT[:, m, ts(nn, NCH)], qp,
                                         AF.Identity, bias=cbias['qb_c'][:, m:m + 1])
                else:
                    nc.scalar.copy(QT[:, m, ts(nn, NCH)], qp)
                kp = psum()
                for k in range(KD):
                    nc.tensor.matmul(kp, wk[:, k, :], rhs_chunk(T4, nn, k),
                                     start=(k == 0), stop=(k == KD - 1))
                if flags['kb']:
                    nc.vector.tensor_scalar_add(KT[:, m, ts(nn, NCH)], kp,
                                                cbias['kb_c'][:, m:m + 1])
                else:
                    nc.vector.tensor_copy(KT[:, m, ts(nn, NCH)], kp)
        for nn in range(2):
            wv = qk.tile([P, KD, NCH], BF16, tag="wv")
            nc.sync.dma_start(wv[:], w['wv_r'][nn])
            for t in range(ST):
                vp_ = psum()
                for k in range(KD):
                    nc.tensor.matmul(vp_, T4[:, t, k, :], wv[:, k, :],
                                     start=(k == 0), stop=(k == KD - 1 and not flags['vb']))
                if flags['vb']:
                    bias_mm(vp_, rows['vb_row'], ts(nn, NCH))
                heads = VP[:, t, :].rearrange("p (h e) -> p h e", e=DK + 1)
                nc.vector.tensor_copy(
                    heads[:, ts(nn, 8), 0:DK],
                    vp_.rearrange("p (h e) -> p h e", e=DK))
        # (QT/KT/wv freed after the score loops below)

        def sa_head(h):
            m, base = divmod(h * DK, P)
            hsl = slice(base, base + DK)
            for nn in range(2):
                expT = expp.tile([P, KD, NCH], BF16, tag="expT")
                for c in range(KD):
                    sp = psum()
                    nc.tensor.matmul(sp, KT[hsl, m, ts(c, P)],
                                     QT[hsl, m, ts(nn, NCH)],
                                     start=True, stop=True,
                                     tile_position=(base, 0))
                    nc.scalar.activation(expT[:, c, :], sp, AF.Exp,
                                         scale=SCALE_SA)
                av = psum(DK + 1, NCH)
                for c in range(KD):
                    nc.tensor.matmul(av, VP[:, c, h * (DK + 1):(h + 1) * (DK + 1)],
                                     expT[:, c, :],
                                     start=(c == 0), stop=(c == KD - 1))
                recip = rbp.tile([1, NCH], F32, tag="recip")
                nc.vector.reciprocal(recip[:], av[DK:DK + 1, :])
                nc.vector.tensor_scalar_mul(recip[:], recip[:],
                                            imp_sa[0:1, h:h + 1])
                rb = rbp.tile([DK, NCH], F32, tag="rb")
                nc.gpsimd.partition_broadcast(rb[:], recip[:])
                nc.vector.tensor_tensor(ctxT[hsl, m, ts(nn, NCH)],
                                        av[0:DK, :], rb[:], OP.mult)

        def ta_head(h):
            m, base = divmod(h * TDK, P)
            hsl = slice(base, base + TDK)
            for nn in range(2):
                expT = expp.tile([P, KD, NCH], BF16, tag="expT")
                for c in range(KD):
                    sp = psum()
                    nc.tensor.matmul(sp, KTt[hsl, m, ts(c, P)],
                                     QTt[hsl, m, ts(nn, NCH)],
                                     start=True, stop=True,
                                     tile_position=(base, 0))
                    nc.scalar.activation(expT[:, c, :], sp, AF.Exp,
                                         scale=SCALE_TA)
                av = psum(TDK + 1, NCH)
                for c in range(KD):
                    nc.tensor.matmul(av, VPt[:, c, h * (TDK + 1):(h + 1) * (TDK + 1)],
                                     expT[:, c, :],
                                     start=(c == 0), stop=(c == KD - 1))
                recip = rbp.tile([1, NCH], F32, tag="recip")
                nc.vector.reciprocal(recip[:], av[TDK:TDK + 1, :])
                nc.vector.tensor_scalar_mul(recip[:], recip[:],
                                            imp_ta[0:1, h:h + 1])
                rbf = rbp.tile([DK, NCH], F32, tag="rb", name="rb")
                rb = rbf[0:TDK, :]
                nc.gpsimd.partition_broadcast(rb[:], recip[:])
                nc.vector.tensor_tensor(ctxTt[hsl, m, ts(nn, NCH)],
                                        av[0:TDK, :], rb[:], OP.mult)

        dbg_dump(nc, "QT", QT[:])
        dbg_dump(nc, "KT", KT[:])
        dbg_dump(nc, "VP", VP[:])
        dbg_dump(nc, "QTt", QTt[:])
        dbg_dump(nc, "KTt", KTt[:])
        dbg_dump(nc, "VPt", VPt[:])

        for h in range(H):
            sa_head(h)
            if h < TH:
                ta_head(h)

        dbg_dump(nc, "ctxT", ctxT[:])
        dbg_dump(nc, "ctxTt", ctxTt[:])
        QKp.close()  # frees QT/KT/wv space

        # out-proj + residual (self), tactical out-proj -> tf
        OPp = ExitStack()
        op = OPp.enter_context(tc.tile_pool(name="op", bufs=2))
        for nn in range(2):
            owh = op.tile([P, KD, NCH], BF16, tag="owh")
            nc.sync.dma_start(owh[:], w['ow_r'][nn])
            for t in range(ST):
                pp = psum()
                for c in range(KD):
                    nc.tensor.matmul(pp, ctxT[:, c, ts(t, P)], owh[:, c, :],
                                     start=(c == 0),
                                     stop=(c == KD - 1 and not flags['ob']))
                if flags['ob']:
                    bias_mm(pp, rows['ob_row'], ts(nn, NCH))
                xs = x_sb[:, t, ts(nn, NCH)]
                nc.vector.tensor_tensor(xs, xs, pp, OP.add)
        tow = op.tile([P, KTD, TD], BF16, tag="tow")
        nc.sync.dma_start(tow[:], w['tow_r'][0])
        for t in range(ST):
            pp = psum(P, TD)
            for c in range(KTD):
                nc.tensor.matmul(pp, ctxTt[:, c, ts(t, P)], tow[:, c, :],
                                 start=(c == 0),
                                 stop=(c == KTD - 1 and not flags['tob']))
            if flags['tob']:
                bias_mm(pp, rows['tob_row'], slice(0, TD))
            nc.scalar.copy(tfrow[:, t, :], pp)
        for t in range(ST):
            nc.sync.dma_start_transpose(tfT[:, t, :, :], tfrow[:, t, :])
        dbg_dump(nc, "x2", x_sb[:])
        dbg_dump(nc, "tf", tfrow[:])
        OPp.close()
        P1.close()

        # ---- phase 2: gate fusion + LN2 ----
        P2 = ExitStack()
        p2 = P2.enter_context(tc.tile_pool(name="p2", bufs=1))
        p2w = P2.enter_context(tc.tile_pool(name="p2w", bufs=2))
        # x2 cast + transpose (reuse rowtmp/T4 slots)
        rowtmp2 = big.tile([P, ST, D], BF16, tag="rowtmp")
        nc.vector.tensor_copy(rowtmp2[:], x_sb[:])
        T4b = big.tile([P, ST, KD, P], BF16, tag="T4")
        for t in range(ST):
            nc.sync.dma_start_transpose(T4b[:, t, :, :], rowtmp2[:, t, :])

        gate = p2.tile([P, ST, D], BF16, tag="gate")
        tproj = p2.tile([P, ST, D], BF16, tag="tproj")
        for nn in range(2):
            tgh = p2w.tile([P, KD + KTD, NCH], BF16, tag="tgh")
            nc.sync.dma_start(tgh[:], w['tg_r'][nn])
            tph = p2w.tile([P, KTD, NCH], BF16, tag="tph")
            nc.sync.dma_start(tph[:], w['tp_r'][nn])
            for t in range(ST):
                gp = psum()
                for c in range(KD + KTD):
                    lhs = T4b[:, t, c, :] if c < KD else tfT[:, t, c - KD, :]
                    nc.tensor.matmul(gp, lhs, tgh[:, c, :], start=(c == 0),
                                     stop=(c == KD + KTD - 1 and not flags['tgb']))
                if flags['tgb']:
                    bias_mm(gp, rows['tgb_row'], ts(nn, NCH))
                nc.scalar.activation(gate[:, t, ts(nn, NCH)], gp, AF.Sigmoid)
                tp_ = psum()
                for c in range(KTD):
                    nc.tensor.matmul(tp_, tfT[:, t, c, :], tph[:, c, :],
                                     start=(c == 0),
                                     stop=(c == KTD - 1 and not flags['tpb']))
                if flags['tpb']:
                    bias_mm(tp_, rows['tpb_row'], ts(nn, NCH))
                nc.scalar.copy(tproj[:, t, ts(nn, NCH)], tp_)
        for t in range(ST):
            nc.vector.tensor_tensor(gate[:, t, :], gate[:, t, :],
                                    tproj[:, t, :], OP.mult)
            nc.vector.tensor_tensor(x_sb[:, t, :], x_sb[:, t, :],
                                    gate[:, t, :], OP.add)
        dbg_dump(nc, "gate_tproj", gate[:])
        dbg_dump(nc, "fused", x_sb[:])
        # LN2 (in place on x_sb, f32)
        _layernorm_rowtiles(nc, lnp, x_sb[:], x_sb[:], eps_ap)
        if flags['g2'] or flags['b2']:
            gb = p2.tile([P, D], F32, tag="g2b")
            bb = p2.tile([P, D], F32, tag="b2b")
            if flags['g2']:
                nc.gpsimd.partition_broadcast(gb[:], rows['g2_r'][:])
            if flags['b2']:
                nc.gpsimd.partition_broadcast(bb[:], rows['b2_r'][:])
            for t in range(ST):
                if flags['g2']:
                    nc.vector.tensor_tensor(x_sb[:, t, :], x_sb[:, t, :],
                                            gb[:], OP.mult)
                if flags['b2']:
                    nc.vector.tensor_tensor(x_sb[:, t, :], x_sb[:, t, :],
                                            bb[:], OP.add)
        P2.close()

        # ---- phase 3: FFN ----
        P3 = ExitStack()
        p3 = P3.enter_context(tc.tile_pool(name="p3", bufs=1))
        rowtmp3 = big.tile([P, ST, D], BF16, tag="rowtmp")
        _layernorm_rowtiles(nc, lnp, x_sb[:], rowtmp3[:], eps_ap)
        T4c = big.tile([P, ST, KD, P], BF16, tag="T4")
        for t in range(ST):
            nc.sync.dma_start_transpose(T4c[:, t, :, :], rowtmp3[:, t, :])

        dbg_dump(nc, "x3", x_sb[:])
        h1T = p3.tile([P, MF, S], BF16, tag="h1T")
        for m in range(MF):
            w1 = wl.tile([P, KD, P], BF16, tag="wl")
            nc.sync.dma_start(w1[:], w['w1_l'][m])
            for nn in range(2):
                hp = psum()
                for k in range(KD):
                    nc.tensor.matmul(hp, w1[:, k, :], rhs_chunk(T4c, nn, k),
                                     start=(k == 0), stop=(k == KD - 1))
                nc.scalar.activation(h1T[:, m, ts(nn, NCH)], hp, AF.Relu,
                                     bias=cbias['h1b_c'][:, m:m + 1])
        for nn in range(2):
            w2h = p3.tile([P, MF, NCH], BF16, tag="w2h")
            nc.sync.dma_start(w2h[:], w['w2_r'][nn])
            for t in range(ST):
                pp = psum()
                for kf in range(MF):
                    nc.tensor.matmul(pp, h1T[:, kf, ts(t, P)], w2h[:, kf, :],
                                     start=(kf == 0),
                                     stop=(kf == MF - 1 and not flags['fb2']))
                if flags['fb2']:
                    bias_mm(pp, rows['fb2_row'], ts(nn, NCH))
                yt = ypool.tile([P, NCH], F32, tag="yt")
                nc.vector.tensor_tensor(yt[:], x_sb[:, t, ts(nn, NCH)], pp,
                                        OP.add)
                nc.sync.dma_start(
                    y_d.rearrange("(t p) d -> p t d", p=P)[:, t, ts(nn, NCH)],
                    yt[:])
        P3.close()

    nc.compile()
    return nc, set(w.keys()) | {"x", "tc"}


_CACHE = {}
LAST_RESULTS = None


def _get_program(params):
    key = "prog"
    if key not in _CACHE:
        d, flags = _pack_weights(params)
        nc, names = build(flags)
        _CACHE[key] = (nc, d, names)
    return _CACHE[key]


def kernel(x, tactical_context, params):
    global LAST_RESULTS
    from concourse.bass_utils import run_bass_kernel_spmd

    x = np.asarray(x, np.float32)
    tctx = np.asarray(tactical_context, np.float32)
    nc, wd, names = _get_program(params)

    in_maps = []
    for c in range(N_CORES):
        m = dict(wd)
        m['x'] = np.ascontiguousarray(x[c])
        m['tc'] = np.ascontiguousarray(tctx[c])
        in_maps.append(m)

    res = run_bass_kernel_spmd(nc, in_maps, core_ids=list(range(N_CORES)))
    LAST_RESULTS = res
    return np.stack([r['y'] for r in res.results]).astype(np.float32)
